# revision 23
# baseline (speedup 1.0000x reference)
"""Block-local attention + FFN Trainium2 kernel (8 NeuronCores, SPMD).

v3: engine-load rebalance. Channels on partitions, tokens on the free dim,
bf16 matmul datapath with f32 PSUM accumulate.

Key changes vs v2:
- QKV/FFN1 weight columns are CENTERED on the host (W~ = W - colmean), so
  W~^T x == W^T (x - mean(x)): the LayerNorm mean subtraction vanishes and
  the LN apply is a single rstd multiply (one DVE op, half the broadcast).
- LN stats tail reads PSUM directly (no staging copy); rstd = exp(-.5 ln v).
- Softmax denominator: Ln reads the ones-column row straight out of PSUM
  (the Ln IS the drain), then a tiny DMA spreads it to 4 partitions and one
  Exp produces 1/d. The 1.1us staging copies of v2 are gone.
- 1/d broadcast to [128, KC, TOK] via stride-0 partition DMA (no PE matmul,
  no PSUM drain); attention normalize is 2 fat DVE ops instead of 8.
- PSUM->SBUF drains split between ACT and DVE to balance engine load.
"""

import numpy as np
import ml_dtypes

import concourse.bass as bass
import concourse.mybir as mybir
import concourse.tile as tile

F32 = mybir.dt.float32
F32R = mybir.dt.float32r
BF16 = mybir.dt.bfloat16
AF = mybir.ActivationFunctionType
ALU = mybir.AluOpType

# Problem constants (hardcoded per the harness contract).
B, C, T, H, W = 2, 512, 8, 32, 32
BT, BH, BW = 4, 8, 8                 # block dims (t, h, w)
NH, DA = 8, 64
EPS = 1e-5
ST, SH, SW = T // BT, H // BH, W // BW
THW = BT * BH * BW                   # 256 tokens per block
NB = B * ST * SH * SW                # 64 blocks
NCORES = 8
NBLK = NB // NCORES                  # 8 blocks per core
KC = C // 128                        # 4 channel chunks
TOK = THW                            # 256

BF = ml_dtypes.bfloat16


def _rep(ap2d, n):
    """Repeat a [P, F] AP n times along a new middle free dim (stride 0)."""
    return bass.AP(tensor=ap2d.tensor, offset=ap2d.offset,
                   ap=[ap2d.ap[0], [0, n], ap2d.ap[1]])


def _bcast_part(ap_row, n):
    """Broadcast a [1, F] AP across n partitions (stride-0 partition dim,
    DMA source only)."""
    return bass.AP(tensor=ap_row.tensor, offset=ap_row.offset,
                   ap=[ap_row.ap[0], [0, n], ap_row.ap[1]])


def _legalize_waits(nc, limit=1):
    """This container's walrus rejects instructions carrying more than ~2
    sem-wait commands (setupSyncWait: "Too many sync wait commands"). Hoist
    excess waits onto preceding single-wait NOPs on the same engine."""
    for f in nc.m.functions:
        for blk in f.blocks:
            newl = []
            changed = False
            for ins in blk.instructions:
                si = ins.sync_info
                waits = list(si.on_wait) if (si is not None and si.on_wait) else []
                if len(waits) > limit:
                    changed = True
                    for k in range(0, len(waits), limit):
                        nop = mybir.InstNoOp(
                            name=f"{ins.name}-ws{k}",
                            sync_info=mybir.SyncInfo(
                                on_wait=list(waits[k:k + limit]), on_update=[]),
                            bass_nofuse=True,
                            engine=ins.engine,
                        )
                        try:
                            nc.register_instruction(nop, overwrite=True)
                        except Exception:
                            pass
                        newl.append(nop)
                    si.on_wait = []
                newl.append(ins)
            if changed:
                try:
                    blk.instructions = newl
                except Exception:
                    blk.instructions.clear()
                    for i in newl:
                        blk.instructions.append(i)


def build_kernel(bq_nz, bk_nz, bv_nz, b1_nz, b2_nz, repeat=1):
    nc = bass.Bass()

    xs_d = nc.declare_dram_parameter("xs", [NBLK, KC, 128, TOK], BF16, isOutput=False)
    wq_d = nc.declare_dram_parameter("wq", [KC, 128, 512], BF16, isOutput=False)
    wk_d = nc.declare_dram_parameter("wk", [KC, 128, 512], BF16, isOutput=False)
    wv_d = nc.declare_dram_parameter("wv", [KC, 128, 512], BF16, isOutput=False)
    wp_d = nc.declare_dram_parameter("wp", [KC, 128, 512], BF16, isOutput=False)
    w1_d = nc.declare_dram_parameter("w1", [KC, 128, 512], BF16, isOutput=False)
    w2_d = nc.declare_dram_parameter("w2", [KC, 128, 512], BF16, isOutput=False)
    ut_d = nc.declare_dram_parameter("ut", [NH, 20, TOK], BF16, isOutput=False)
    vt_d = nc.declare_dram_parameter("vt", [NH, 20, TOK], BF16, isOutput=False)
    br_d = nc.declare_dram_parameter("brows", [128, 16], F32, isOutput=False)
    bv_d = nc.declare_dram_parameter("bvrow", [1, 512], BF16, isOutput=False)
    sel_d = nc.declare_dram_parameter("sel", [NH, NH * 64], BF16, isOutput=False)
    out_d = nc.declare_dram_parameter("out", [NBLK, KC, 128, TOK], F32, isOutput=True)

    from contextlib import ExitStack

    with nc.allow_low_precision(reason="bf16 matmul/elementwise datapath"), \
            tile.TileContext(nc) as tc, ExitStack() as ctx:
        cp = ctx.enter_context(tc.tile_pool(name="const", bufs=1))
        sb = ctx.enter_context(tc.tile_pool(name="sb", bufs=2))
        sbe = ctx.enter_context(tc.tile_pool(name="sbe", bufs=3))
        lnp = ctx.enter_context(tc.tile_pool(name="lnp", bufs=3))
        ps = ctx.enter_context(tc.tile_pool(name="ps", bufs=2, space="PSUM"))
        pss = ctx.enter_context(tc.tile_pool(name="pss", bufs=2, space="PSUM"))
        psa = ctx.enter_context(tc.tile_pool(name="psa", bufs=1, space="PSUM"))

        # --- persistent constants ---
        wq_s = cp.tile([128, KC, 512], BF16)
        wk_s = cp.tile([128, KC, 512], BF16)
        wv_s = cp.tile([128, KC, 512], BF16)
        wp_s = cp.tile([128, KC, 512], BF16)
        w1_s = cp.tile([128, KC, 512], BF16)
        w2_s = cp.tile([128, KC, 512], BF16)
        for w_s, w_d in ((wq_s, wq_d), (wk_s, wk_d), (wv_s, wv_d),
                         (wp_s, wp_d), (w1_s, w1_d), (w2_s, w2_d)):
            for kc in range(KC):
                nc.gpsimd.dma_start(w_s[:, kc, :], w_d[kc])
        ut_s = cp.tile([20, NH, TOK], BF16)
        vt_s = cp.tile([20, NH, TOK], BF16)
        for hh in range(NH):
            nc.gpsimd.dma_start(ut_s[:, hh, :], ut_d[hh])
            nc.gpsimd.dma_start(vt_s[:, hh, :], vt_d[hh])
        sel = cp.tile([NH, NH * 64], BF16)
        nc.gpsimd.dma_start(sel[:], sel_d[:])
        br_s = cp.tile([128, 16], F32)
        nc.gpsimd.dma_start(br_s[:], br_d[:])
        bvr_s = cp.tile([1, 512], BF16)
        nc.gpsimd.dma_start(bvr_s[0:1, :], bv_d[:])
        ones16f = cp.tile([128, 16], F32)
        nc.vector.memset(ones16f[:], 1.0)
        ones16b = cp.tile([128, 16], BF16)
        nc.scalar.activation(ones16b[:], ones16f[:], AF.Copy)
        onesc = cp.tile([128, 1], BF16)          # 1/C for mean via matmul
        nc.scalar.activation(onesc[:], ones16f[:, 0:1], AF.Copy, scale=1.0 / C)
        # [1/C, 0] and [0, 1/C] stationaries: two stats matmuls accumulate
        # into rows 0/1 of one PSUM tile (output base partition must be 0).
        oneszf = cp.tile([128, 2], F32)
        nc.vector.memset(oneszf[:], 0.0)
        onescA = cp.tile([128, 2], BF16)
        nc.scalar.activation(onescA[:], oneszf[:], AF.Copy)
        nc.scalar.activation(onescA[:, 0:1], ones16f[:, 0:1], AF.Copy,
                             scale=1.0 / C)
        onescB = cp.tile([128, 2], BF16)
        nc.scalar.activation(onescB[:], oneszf[:], AF.Copy)
        nc.scalar.activation(onescB[:, 1:2], ones16f[:, 0:1], AF.Copy,
                             scale=1.0 / C)
        onesrf = cp.tile([1, 128], F32)
        nc.vector.memset(onesrf[0:1, :], 1.0)
        ones_row = cp.tile([1, 128], BF16)
        nc.scalar.activation(ones_row[0:1, :], onesrf[0:1, :], AF.Copy)

        # Two LayerNorm stats (LN1 of block t+1, LN2 of block t-1) that land
        # in the same pipeline iteration share one PSUM tile and one set of
        # tail ops on [2, 256] — halves the ACT/DVE small-op overhead.
        stats_ctx = {"tile": None, "pend": []}

        def _ln_pre(src, st, key, tag, expect_pair=False):
            """Pre-reduce src and matmul [sum | sumsq] into a shared row."""
            if stats_ctx["tile"] is None:
                ps_ln = pss.tile([128, 512], F32, tag="s")
                stats_ctx["tile"] = ps_ln
            row = len(stats_ctx["pend"])
            sq = lnp.tile([128, KC, TOK], BF16, tag=f"sq{tag}")
            nc.vector.tensor_mul(sq[:], src[:], src[:])
            t4 = lnp.tile([128, 2, 2, TOK], BF16, tag=f"t4{tag}")
            nc.vector.tensor_add(t4[:, 0], src[:, 0:2, :], src[:, 2:4, :])
            nc.vector.tensor_add(t4[:, 1], sq[:, 0:2, :], sq[:, 2:4, :])
            red = lnp.tile([128, 2, TOK], BF16, tag=f"red{tag}")
            nc.vector.tensor_add(red[:], t4[:, :, 0, :], t4[:, :, 1, :])
            nc.tensor.matmul(stats_ctx["tile"][0:2, :],
                             onescA[:] if row == 0 else onescB[:],
                             red[:], start=(row == 0),
                             stop=(row == 1 or not expect_pair),
                             skip_group_check=True)
            stats_ctx["pend"].append((st, key))

        def _ln_flush():
            """Run the stats tail on all pending rows, stash rstd rows."""
            n = len(stats_ctx["pend"])
            if n == 0:
                return
            ps_ln = stats_ctx["tile"]
            m2 = lnp.tile([2, 256], F32, tag="m2")
            nc.scalar.activation(m2[0:n, :], ps_ln[0:n, 0:256], AF.Square)
            var = lnp.tile([2, 256], F32, tag="var")
            nc.vector.scalar_tensor_tensor(var[0:n, :], ps_ln[0:n, 256:512],
                                           EPS, m2[0:n, :],
                                           op0=ALU.add, op1=ALU.subtract)
            lnv = lnp.tile([2, 256], F32, tag="lnv")
            nc.scalar.activation(lnv[0:n, :], var[0:n, :], AF.Ln)
            rstd = lnp.tile([2, 256], BF16, tag="rstd")
            # rstd = exp(-0.5 * ln(var)) — keeps ACT on the exp/ln table.
            nc.scalar.activation(rstd[0:n, :], lnv[0:n, :], AF.Exp,
                                 scale=-0.5)
            for row, (st, key) in enumerate(stats_ctx["pend"]):
                st[key] = (rstd, row)
            stats_ctx["tile"] = None
            stats_ctx["pend"] = []

        def _ln_bcast(rstd_row, tag):
            """Broadcast an rstd row across 128 partitions (stride-0 DMA)."""
            rstd, row = rstd_row
            rb = sbe.tile([128, 256], BF16, tag=f"rb{tag}")
            nc.scalar.dma_start(rb[:], _bcast_part(rstd[row:row + 1, :], 128))
            return rb

        def _apply(src, rb, dst_tag):
            """xhat = src * rstd (weights are host-centered, no mean term)."""
            dst = sb.tile([128, KC, TOK], BF16, tag=dst_tag)
            nc.vector.tensor_mul(dst[:], src[:], _rep(rb[:, :], KC))
            return dst

        def pf_load(t):
            st = {"b": t}
            x_sb = sbe.tile([128, KC, TOK], BF16, tag="x_sb")
            nc.sync.dma_start(x_sb[:],
                              xs_d[t % NBLK].rearrange("a p b -> p a b"))
            st["x"] = x_sb
            return st

        def s0a_stats(st, expect_pair=False):
            _ln_pre(st["x"], st, "rstd1", "1", expect_pair=expect_pair)

        def s0b_bcast(st):
            st["rb1"] = _ln_bcast(st["rstd1"], "1")

        def s1_apply(st):
            st["xh"] = _apply(st["x"], st["rb1"], "xhat1")

        def s1_qkv(st):
            xh = st.pop("xh")
            qT = sb.tile([128, KC, TOK], BF16, tag="qT")
            kT = sb.tile([128, KC, TOK], BF16, tag="kT")
            for dst, w_s, bcol0, nz, dve in ((qT, wq_s, 0, bq_nz, True),
                                             (kT, wk_s, 4, bk_nz, False)):
                for pair in range(2):
                    ps_q = ps.tile([128, 512], F32, tag="mm")
                    for half in range(2):
                        mf = pair * 2 + half
                        o = ps_q[:, half * 256:(half + 1) * 256]
                        for kc in range(KC):
                            nc.tensor.matmul(
                                o, w_s[:, kc, mf * 128:(mf + 1) * 128],
                                xh[:, kc, :],
                                start=(kc == 0), stop=(kc == KC - 1))
                    if nz:
                        for half in range(2):
                            mf = pair * 2 + half
                            nc.scalar.activation(
                                dst[:, mf, :],
                                ps_q[:, half * 256:(half + 1) * 256],
                                AF.Identity, bias=br_s[:, bcol0 + mf:bcol0 + mf + 1])
                    elif dve:
                        nc.vector.tensor_copy(
                            dst[:, pair * 2:(pair + 1) * 2, :],
                            ps_q[:].rearrange("p (a b) -> p a b", a=2))
                    else:
                        nc.scalar.activation(
                            dst[:, pair * 2:(pair + 1) * 2, :],
                            ps_q[:].rearrange("p (a b) -> p a b", a=2), AF.Copy)
            v65 = sb.tile([128, 2, NH, 65], BF16, tag="v65")
            nc.scalar.activation(
                v65[:, :, :, 64:65],
                ones16b[:].rearrange("p (a h b) -> p a h b", a=2, h=NH), AF.Copy)
            for tcx in range(2):
                ps_v = ps.tile([128, 512], F32, tag="mm")
                for kc in range(KC):
                    nc.tensor.matmul(
                        ps_v[:], xh[:, kc, tcx * 128:(tcx + 1) * 128],
                        wv_s[:, kc, :],
                        start=(kc == 0), stop=(kc == KC - 1 and not bv_nz))
                if bv_nz:
                    nc.tensor.matmul(ps_v[:], ones_row[:], bvr_s[0:1, :],
                                     start=False, stop=True)
                if tcx == 0:
                    nc.vector.tensor_copy(
                        v65[:, tcx, :, 0:64],
                        ps_v[:].rearrange("p (h e) -> p h e", h=NH))
                else:
                    nc.scalar.activation(
                        v65[:, tcx, :, 0:64],
                        ps_v[:].rearrange("p (h e) -> p h e", h=NH), AF.Copy)
            st["qT"], st["kT"], st["v65"] = qT, kT, v65

        def _head(st, hh):
            """Scores (+ low-rank bias) -> exp. Returns e_t for the AV step."""
            qT, kT = st["qT"], st["kT"]
            mf, po = hh // 2, (hh % 2) * 64
            ps_s = pss.tile([128, 512], F32, tag="s")
            for kt in range(2):
                nc.tensor.matmul(
                    ps_s[:, kt * 256:(kt + 1) * 256],
                    kT[po:po + 64, mf, kt * 128:(kt + 1) * 128],
                    qT[po:po + 64, mf, :], start=True, stop=False)
                nc.tensor.matmul(
                    ps_s[:, kt * 256:(kt + 1) * 256],
                    vt_s[:, hh, kt * 128:(kt + 1) * 128],
                    ut_s[:, hh, :], start=False, stop=True)
            e_t = lnp.tile([128, 2, TOK], BF16, tag="E")
            nc.scalar.activation(e_t[:],
                                 ps_s[:].rearrange("p (a b) -> p a b", a=2),
                                 AF.Exp)
            return e_t

        def _head_av(st, hh, e_t):
            v65 = st["v65"]
            for kt in range(2):
                nc.tensor.matmul(st["ps_av"][:, hh, :], v65[:, kt, hh, :],
                                 e_t[:, kt, :],
                                 start=(kt == 0), stop=(kt == 1))
            if hh % 4 == 3:
                # Half-group denominator chain: Ln reads the ones-column row
                # straight from PSUM (the Ln IS the drain), a tiny DMA
                # spreads it to 4 partitions, one Exp gives 1/d. The first
                # half completes while heads 4-7 are still on the PE.
                g = hh // 4
                d8l = lnp.tile([1, 4, TOK], F32, tag="d8l")
                nc.scalar.activation(d8l[0:1, :, :],
                                     st["ps_av"][64:65, hh - 3:hh + 1, :],
                                     AF.Ln)
                d8f = sbe.tile([4, TOK], F32, tag=f"d8f{g}")
                nc.scalar.dma_start(d8f[:], d8l[0:1, :, :])
                d8r = sbe.tile([4, TOK], BF16, tag=f"d8r{g}")
                nc.scalar.activation(d8r[:], d8f[:], AF.Exp, scale=-1.0)
                st[f"d8r{g}"] = d8r

        def _norm_group(st, g):
            """Broadcast group g's 1/denom via the sel matmul and normalize
            its 4 heads into aT chunks 2g..2g+1 (two [64, 2, TOK] DVE ops).
            Group 0 runs inside s2b, overlapped with heads 5-7 on the PE."""
            ps_av, rb8, aT = st["ps_av"], st["rb8"], st["aT"]
            d8r = st[f"d8r{g}"]
            ps_rb = pss.tile([128, 512], F32, tag="s")
            for half in range(2):
                nc.tensor.matmul(ps_rb[:, half * 256:(half + 1) * 256],
                                 sel[0:4, half * 128:(half + 1) * 128],
                                 d8r[:, :], start=True, stop=True)
            nc.vector.tensor_copy(
                rb8[:, g * 2:(g + 1) * 2, :],
                ps_rb[:].rearrange("p (a b) -> p a b", a=2))
            c0, c1 = 2 * g, 2 * g + 2
            nc.vector.tensor_mul(aT[0:64, c0:c1, :],
                                 ps_av[0:64, 4 * g:4 * g + 4:2, :],
                                 rb8[0:64, c0:c1, :])
            nc.vector.tensor_mul(aT[64:128, c0:c1, :],
                                 ps_av[0:64, 4 * g + 1:4 * g + 4:2, :],
                                 rb8[64:128, c0:c1, :])

        def s2a_attn(st):
            """Heads 0-3, software-pipelined by one head."""
            ps_av = psa.tile([65, NH, TOK], F32, tag="av")
            rb8 = sbe.tile([128, KC, TOK], BF16, tag="rb8")
            aT = sb.tile([128, KC, TOK], BF16, tag="aT")
            st["ps_av"], st["rb8"], st["aT"] = ps_av, rb8, aT
            e_prev = _head(st, 0)
            for hh in range(1, 4):
                e_t = _head(st, hh)
                _head_av(st, hh - 1, e_prev)
                e_prev = e_t
            st["e_prev"] = e_prev

        def s2b_attn(st):
            e_prev = st.pop("e_prev")
            for hh in range(4, 8):
                e_t = _head(st, hh)
                _head_av(st, hh - 1, e_prev)
                e_prev = e_t
                if hh == 7:
                    # Group 0's normalize: its reciprocal chain finished
                    # while heads 5-6 were on the PE.
                    _norm_group(st, 0)
            _head_av(st, 7, e_prev)

        def s3a_norm_proj(st):
            _norm_group(st, 1)
            aT = st["aT"]
            o_sb = sbe.tile([128, KC, TOK], BF16, tag="o_sb")
            for pair in range(2):
                ps_o = ps.tile([128, 512], F32, tag="mm")
                for half in range(2):
                    mc = pair * 2 + half
                    o = ps_o[:, half * 256:(half + 1) * 256]
                    for fc in range(KC):
                        nc.tensor.matmul(
                            o, wp_s[:, fc, mc * 128:(mc + 1) * 128],
                            aT[:, fc, :],
                            start=(fc == 0), stop=(fc == KC - 1))
                nc.vector.tensor_add(
                    o_sb[:, pair * 2:(pair + 1) * 2, :],
                    ps_o[:].rearrange("p (a b) -> p a b", a=2),
                    st["x"][:, pair * 2:(pair + 1) * 2, :])
            st["o"] = o_sb

        def s3b1_stats(st):
            _ln_pre(st["o"], st, "rstd2", "2")

        def s3b2_bcast(st):
            st["rb2"] = _ln_bcast(st["rstd2"], "2")

        def s4a_apply(st):
            st["yh"] = _apply(st["o"], st["rb2"], "xhat2")

        def s4a_ffn1(st):
            yh = st.pop("yh")
            h1 = sb.tile([128, KC, TOK], BF16, tag="h1")
            for pair in range(2):
                ps_h = ps.tile([128, 512], F32, tag="mm")
                for half in range(2):
                    mf = pair * 2 + half
                    o = ps_h[:, half * 256:(half + 1) * 256]
                    for kc in range(KC):
                        nc.tensor.matmul(
                            o, w1_s[:, kc, mf * 128:(mf + 1) * 128],
                            yh[:, kc, :],
                            start=(kc == 0), stop=(kc == KC - 1))
                if b1_nz:
                    for half in range(2):
                        mf = pair * 2 + half
                        nc.scalar.activation(
                            h1[:, mf, :], ps_h[:, half * 256:(half + 1) * 256],
                            AF.Relu, bias=br_s[:, 8 + mf:8 + mf + 1])
                else:
                    nc.scalar.activation(
                        h1[:, pair * 2:(pair + 1) * 2, :],
                        ps_h[:].rearrange("p (a b) -> p a b", a=2), AF.Relu)
            st["h1"] = h1

        def s4b_ffn2(st):
            o_sb, h1 = st["o"], st["h1"]
            out_sb = sb.tile([128, KC, TOK], F32, tag="out_sb")
            for pair in range(2):
                ps_y = ps.tile([128, 512], F32, tag="mm")
                for half in range(2):
                    mc = pair * 2 + half
                    o = ps_y[:, half * 256:(half + 1) * 256]
                    for fc in range(KC):
                        nc.tensor.matmul(
                            o, w2_s[:, fc, mc * 128:(mc + 1) * 128],
                            h1[:, fc, :],
                            start=(fc == 0), stop=(fc == KC - 1))
                if b2_nz:
                    for half in range(2):
                        mc = pair * 2 + half
                        nc.vector.scalar_tensor_tensor(
                            out_sb[:, mc, :],
                            ps_y[:, half * 256:(half + 1) * 256],
                            br_s[:, 12 + mc:12 + mc + 1],
                            o_sb[:, mc, :], op0=ALU.add, op1=ALU.add)
                else:
                    nc.vector.tensor_add(
                        out_sb[:, pair * 2:(pair + 1) * 2, :],
                        ps_y[:].rearrange("p (a b) -> p a b", a=2),
                        o_sb[:, pair * 2:(pair + 1) * 2, :])
            nc.gpsimd.dma_start(out_d[st["b"] % NBLK].rearrange("a p b -> p a b"),
                                out_sb[:])

        # Software pipeline across blocks. PE order per iteration:
        # scores/AV(t-1) -> QKV(t) -> ffn1(t-2) -> selMM+proj(t-1) ->
        # ffn2(t-2): the t-1 softmax-denominator chain (ACT/DMA/DVE) hides
        # under ~6us of QKV+ffn1 matmuls before proj needs aT.
        blocks = {}
        NT = NBLK * repeat
        blocks[0] = pf_load(0)
        s0a_stats(blocks[0])
        _ln_flush()
        s0b_bcast(blocks[0])
        for t in range(NT):
            if t - 1 >= 0:
                s2a_attn(blocks[t - 1])
            s1_apply(blocks[t])
            if t - 2 >= 0:
                s4a_apply(blocks[t - 2])
            if t - 1 >= 0:
                s2b_attn(blocks[t - 1])
            s1_qkv(blocks[t])
            if t - 2 >= 0:
                s4a_ffn1(blocks[t - 2])
            if t - 1 >= 0:
                s3a_norm_proj(blocks[t - 1])
            if t + 1 < NT:
                blocks[t + 1] = pf_load(t + 1)
                s0a_stats(blocks[t + 1], expect_pair=(t - 1 >= 0))
            if t - 1 >= 0:
                s3b1_stats(blocks[t - 1])
            _ln_flush()
            if t + 1 < NT:
                s0b_bcast(blocks[t + 1])
            if t - 2 >= 0:
                s4b_ffn2(blocks.pop(t - 2))
            if t - 1 >= 0:
                s3b2_bcast(blocks[t - 1])
        tl = NT - 1
        s2a_attn(blocks[tl])
        s4a_apply(blocks[tl - 1])
        s2b_attn(blocks[tl])
        s4a_ffn1(blocks[tl - 1])
        s3a_norm_proj(blocks[tl])
        s3b1_stats(blocks[tl])
        _ln_flush()
        s4b_ffn2(blocks.pop(tl - 1))
        s3b2_bcast(blocks[tl])
        s4a_apply(blocks[tl])
        s4a_ffn1(blocks[tl])
        s4b_ffn2(blocks.pop(tl))

    _legalize_waits(nc)
    return nc


_CACHE = {}


def _get_nc(flags, repeat=1):
    key = (flags, repeat)
    if key not in _CACHE:
        _CACHE[key] = build_kernel(*flags, repeat=repeat)
    return _CACHE[key]


def _axial_bias_np(dt_bank, dh_bank, dw_bank):
    ar = np.arange(THW)
    tt = ar // (BH * BW)
    hh = (ar // BW) % BH
    ww = ar % BW
    it = tt[:, None] - tt[None, :] + (BT - 1)
    ih = hh[:, None] - hh[None, :] + (BH - 1)
    iw = ww[:, None] - ww[None, :] + (BW - 1)
    return dt_bank[:, it] + dh_bank[:, ih] + dw_bank[:, iw]  # (NH, 256, 256)


def prepare(x, dt_bank, dh_bank, dw_bank, ln1_g, ln1_b, w_q, w_k, w_v,
            w_proj, ln2_g, ln2_b, w1, b1, w2, b2):
    """Host-side prep: returns (flags, in_maps)."""
    f = np.float32
    x = np.asarray(x, f)

    # block split: (B,C,T,H,W) -> (NB, C, THW), channels-major
    xb = x.reshape(B, C, ST, BT, SH, BH, SW, BW)
    xb = xb.transpose(0, 2, 4, 6, 1, 3, 5, 7).reshape(NB, C, THW)
    xb = np.ascontiguousarray(xb).reshape(NB, KC, 128, TOK).astype(BF)

    scale = 1.0 / np.sqrt(DA)
    wqf = np.asarray(w_q, f).transpose(1, 0, 2).reshape(C, NH * DA)
    wkf = np.asarray(w_k, f).transpose(1, 0, 2).reshape(C, NH * DA)
    wvf = np.asarray(w_v, f).transpose(1, 0, 2).reshape(C, NH * DA)
    g1 = np.asarray(ln1_g, f)[:, None]
    b1v = np.asarray(ln1_b, f)
    # Center columns (over the contract dim): W~^T x == W^T (x - mean(x)).
    wq_c = (g1 * wqf) * scale
    wk_c = g1 * wkf
    wv_c = g1 * wvf
    wq_c = wq_c - wq_c.mean(axis=0, keepdims=True)
    wk_c = wk_c - wk_c.mean(axis=0, keepdims=True)
    wv_c = wv_c - wv_c.mean(axis=0, keepdims=True)
    wq_e = np.ascontiguousarray(wq_c).reshape(KC, 128, 512)
    wk_e = np.ascontiguousarray(wk_c).reshape(KC, 128, 512)
    wv_e = np.ascontiguousarray(wv_c).reshape(KC, 128, 512)
    bq = (b1v @ wqf) * scale
    bk = b1v @ wkf
    bv = b1v @ wvf
    wp_e = np.ascontiguousarray(np.asarray(w_proj, f).T).reshape(KC, 128, 512)
    g2 = np.asarray(ln2_g, f)[:, None]
    b2v = np.asarray(ln2_b, f)
    w1t = np.asarray(w1, f).T
    w1_c = g2 * w1t
    w1_c = w1_c - w1_c.mean(axis=0, keepdims=True)
    w1_e = np.ascontiguousarray(w1_c).reshape(KC, 128, 512)
    b1p = b2v @ w1t + np.asarray(b1, f)
    w2_e = np.ascontiguousarray(np.asarray(w2, f).T).reshape(KC, 128, 512)
    b2p = np.asarray(b2, f)

    brows = np.zeros((128, 16), f)
    brows[:, 0:4] = bq.reshape(KC, 128).T
    brows[:, 4:8] = bk.reshape(KC, 128).T
    brows[:, 8:12] = b1p.reshape(KC, 128).T
    brows[:, 12:16] = b2p.reshape(KC, 128).T
    bvrow = np.ascontiguousarray(bv.reshape(1, 512)).astype(BF)

    # Low-rank factorization of the (transposed) axial bias: B^T = V @ U^T,
    # exact rank <= 4+8+8 = 20, accumulated into the scores matmul on PE.
    bias = _axial_bias_np(np.asarray(dt_bank, f), np.asarray(dh_bank, f),
                          np.asarray(dw_bank, f))
    R = 20
    ut = np.zeros((NH, R, THW), np.float64)
    vt = np.zeros((NH, R, THW), np.float64)
    for h in range(NH):
        U, S, Vh = np.linalg.svd(bias[h].T.astype(np.float64))
        r = min(R, int((S > S[0] * 1e-7).sum()) if S[0] > 0 else 1)
        vt[h, :r] = (U[:, :r] * np.sqrt(S[:r])).T
        ut[h, :r] = Vh[:r] * np.sqrt(S[:r])[:, None]
    ut = np.ascontiguousarray(ut).astype(BF)
    vt = np.ascontiguousarray(vt).astype(BF)

    flags = (bool(bq.any()), bool(bk.any()), bool(bv.any()),
             bool(b1p.any()), bool(b2p.any()))

    # sel[j, half*128 + p] = 1 iff 2*half + p//64 == j (j < 4): broadcasts a
    # 4-head group's 1/denom rows to all 128 partitions of a channel chunk.
    selm = np.zeros((NH, KC * 128), f)
    for half in range(2):
        for p in range(128):
            selm[2 * half + p // 64, half * 128 + p] = 1.0

    shared = {"wq": wq_e.astype(BF), "wk": wk_e.astype(BF),
              "wv": wv_e.astype(BF), "wp": wp_e.astype(BF),
              "w1": w1_e.astype(BF), "w2": w2_e.astype(BF),
              "ut": ut, "vt": vt, "brows": brows, "bvrow": bvrow,
              "sel": selm.astype(BF)}
    in_maps = []
    for i in range(NCORES):
        m = dict(shared)
        m["xs"] = np.ascontiguousarray(xb[i * NBLK:(i + 1) * NBLK])
        in_maps.append(m)
    return flags, in_maps


def gather(results):
    outs = np.concatenate([results[i]["out"][None] for i in range(NCORES)])
    # (NCORES, NBLK, KC, 128, TOK) -> (NB, C, THW) -> (B, C, T, H, W)
    ob = outs.reshape(NB, C, THW)
    ob = ob.reshape(B, ST, SH, SW, C, BT, BH, BW)
    ob = ob.transpose(0, 4, 1, 5, 2, 6, 3, 7).reshape(B, C, T, H, W)
    return np.ascontiguousarray(ob)


def kernel(**inputs):
    from concourse.bass_utils import run_bass_kernel_spmd

    flags, in_maps = prepare(**inputs)
    nc = _get_nc(flags)
    res = run_bass_kernel_spmd(nc, in_maps, list(range(NCORES)))
    return gather(res.results)


# revision 29
# speedup vs baseline: 1.0862x; 1.0862x over previous
"""Block-local attention + FFN Trainium2 kernel (8 NeuronCores, SPMD).

v3: engine-load rebalance. Channels on partitions, tokens on the free dim,
bf16 matmul datapath with f32 PSUM accumulate.

Key changes vs v2:
- QKV/FFN1 weight columns are CENTERED on the host (W~ = W - colmean), so
  W~^T x == W^T (x - mean(x)): the LayerNorm mean subtraction vanishes and
  the LN apply is a single rstd multiply (one DVE op, half the broadcast).
- LN stats tail reads PSUM directly (no staging copy); rstd = exp(-.5 ln v).
- Softmax denominator: Ln reads the ones-column row straight out of PSUM
  (the Ln IS the drain), then a tiny DMA spreads it to 4 partitions and one
  Exp produces 1/d. The 1.1us staging copies of v2 are gone.
- 1/d broadcast to [128, KC, TOK] via stride-0 partition DMA (no PE matmul,
  no PSUM drain); attention normalize is 2 fat DVE ops instead of 8.
- PSUM->SBUF drains split between ACT and DVE to balance engine load.
"""

import numpy as np
import ml_dtypes

import concourse.bass as bass
import concourse.mybir as mybir
import concourse.tile as tile

F32 = mybir.dt.float32
F32R = mybir.dt.float32r
BF16 = mybir.dt.bfloat16
AF = mybir.ActivationFunctionType
ALU = mybir.AluOpType

# Problem constants (hardcoded per the harness contract).
B, C, T, H, W = 2, 512, 8, 32, 32
BT, BH, BW = 4, 8, 8                 # block dims (t, h, w)
NH, DA = 8, 64
EPS = 1e-5
ST, SH, SW = T // BT, H // BH, W // BW
THW = BT * BH * BW                   # 256 tokens per block
NB = B * ST * SH * SW                # 64 blocks
NCORES = 8
NBLK = NB // NCORES                  # 8 blocks per core
KC = C // 128                        # 4 channel chunks
TOK = THW                            # 256

BF = ml_dtypes.bfloat16


def _rep(ap2d, n):
    """Repeat a [P, F] AP n times along a new middle free dim (stride 0)."""
    return bass.AP(tensor=ap2d.tensor, offset=ap2d.offset,
                   ap=[ap2d.ap[0], [0, n], ap2d.ap[1]])


def _bcast_part(ap_row, n):
    """Broadcast a [1, F] AP across n partitions (stride-0 partition dim,
    DMA source only)."""
    return bass.AP(tensor=ap_row.tensor, offset=ap_row.offset,
                   ap=[ap_row.ap[0], [0, n], ap_row.ap[1]])


def _legalize_waits(nc, limit=1):
    """This container's walrus rejects instructions carrying more than ~2
    sem-wait commands (setupSyncWait: "Too many sync wait commands"). Hoist
    excess waits onto preceding single-wait NOPs on the same engine."""
    for f in nc.m.functions:
        for blk in f.blocks:
            newl = []
            changed = False
            for ins in blk.instructions:
                si = ins.sync_info
                waits = list(si.on_wait) if (si is not None and si.on_wait) else []
                if len(waits) > limit:
                    changed = True
                    for k in range(0, len(waits), limit):
                        nop = mybir.InstNoOp(
                            name=f"{ins.name}-ws{k}",
                            sync_info=mybir.SyncInfo(
                                on_wait=list(waits[k:k + limit]), on_update=[]),
                            bass_nofuse=True,
                            engine=ins.engine,
                        )
                        try:
                            nc.register_instruction(nop, overwrite=True)
                        except Exception:
                            pass
                        newl.append(nop)
                    si.on_wait = []
                newl.append(ins)
            if changed:
                try:
                    blk.instructions = newl
                except Exception:
                    blk.instructions.clear()
                    for i in newl:
                        blk.instructions.append(i)


def build_kernel(bq_nz, bk_nz, bv_nz, b1_nz, b2_nz, repeat=1):
    nc = bass.Bass()

    xs_d = nc.declare_dram_parameter("xs", [NBLK, KC, 128, TOK], BF16, isOutput=False)
    wq_d = nc.declare_dram_parameter("wq", [KC, 128, 512], BF16, isOutput=False)
    wk_d = nc.declare_dram_parameter("wk", [KC, 128, 512], BF16, isOutput=False)
    wv_d = nc.declare_dram_parameter("wv", [KC, 128, 512], BF16, isOutput=False)
    wp_d = nc.declare_dram_parameter("wp", [KC, 128, 512], BF16, isOutput=False)
    w1_d = nc.declare_dram_parameter("w1", [KC, 128, 512], BF16, isOutput=False)
    w2_d = nc.declare_dram_parameter("w2", [KC, 128, 512], BF16, isOutput=False)
    ut_d = nc.declare_dram_parameter("ut", [NH, 20, TOK], BF16, isOutput=False)
    vt_d = nc.declare_dram_parameter("vt", [NH, 20, TOK], BF16, isOutput=False)
    br_d = nc.declare_dram_parameter("brows", [128, 16], F32, isOutput=False)
    bv_d = nc.declare_dram_parameter("bvrow", [1, 512], BF16, isOutput=False)
    sel_d = nc.declare_dram_parameter("sel", [NH, NH * 64], BF16, isOutput=False)
    out_d = nc.declare_dram_parameter("out", [NBLK, KC, 128, TOK], F32, isOutput=True)

    from contextlib import ExitStack

    with nc.allow_low_precision(reason="bf16 matmul/elementwise datapath"), \
            tile.TileContext(nc) as tc, ExitStack() as ctx:
        cp = ctx.enter_context(tc.tile_pool(name="const", bufs=1))
        sb = ctx.enter_context(tc.tile_pool(name="sb", bufs=2))
        sbe = ctx.enter_context(tc.tile_pool(name="sbe", bufs=3))
        lnp = ctx.enter_context(tc.tile_pool(name="lnp", bufs=3))
        # One shared 4-deep pool for every [128, 512] PSUM tile (GEMM outs,
        # scores, stats, sel broadcasts): the dynamic scheduler shares the 4
        # banks across stages. The AV accumulator is two 2-bank group tiles
        # rotating through 2 slots. 4 + 4 = 8 banks.
        pp = ctx.enter_context(tc.tile_pool(name="pp", bufs=4, space="PSUM"))
        psa = ctx.enter_context(tc.tile_pool(name="psa", bufs=2, space="PSUM"))

        # --- persistent constants ---
        wq_s = cp.tile([128, KC, 512], BF16)
        wk_s = cp.tile([128, KC, 512], BF16)
        wv_s = cp.tile([128, KC, 512], BF16)
        wp_s = cp.tile([128, KC, 512], BF16)
        w1_s = cp.tile([128, KC, 512], BF16)
        w2_s = cp.tile([128, KC, 512], BF16)
        for w_s, w_d in ((wq_s, wq_d), (wk_s, wk_d), (wv_s, wv_d),
                         (wp_s, wp_d), (w1_s, w1_d), (w2_s, w2_d)):
            for kc in range(KC):
                nc.gpsimd.dma_start(w_s[:, kc, :], w_d[kc])
        ut_s = cp.tile([20, NH, TOK], BF16)
        vt_s = cp.tile([20, NH, TOK], BF16)
        for hh in range(NH):
            nc.gpsimd.dma_start(ut_s[:, hh, :], ut_d[hh])
            nc.gpsimd.dma_start(vt_s[:, hh, :], vt_d[hh])
        sel = cp.tile([NH, NH * 64], BF16)
        nc.gpsimd.dma_start(sel[:], sel_d[:])
        br_s = cp.tile([128, 16], F32)
        nc.gpsimd.dma_start(br_s[:], br_d[:])
        bvr_s = cp.tile([1, 512], BF16)
        nc.gpsimd.dma_start(bvr_s[0:1, :], bv_d[:])
        ones16f = cp.tile([128, 16], F32)
        nc.vector.memset(ones16f[:], 1.0)
        ones16b = cp.tile([128, 16], BF16)
        nc.scalar.activation(ones16b[:], ones16f[:], AF.Copy)
        onesc = cp.tile([128, 1], BF16)          # 1/C for mean via matmul
        nc.scalar.activation(onesc[:], ones16f[:, 0:1], AF.Copy, scale=1.0 / C)
        # [1/C, 0] and [0, 1/C] stationaries: two stats matmuls accumulate
        # into rows 0/1 of one PSUM tile (output base partition must be 0).
        oneszf = cp.tile([128, 2], F32)
        nc.vector.memset(oneszf[:], 0.0)
        onescA = cp.tile([128, 2], BF16)
        nc.scalar.activation(onescA[:], oneszf[:], AF.Copy)
        nc.scalar.activation(onescA[:, 0:1], ones16f[:, 0:1], AF.Copy,
                             scale=1.0 / C)
        onescB = cp.tile([128, 2], BF16)
        nc.scalar.activation(onescB[:], oneszf[:], AF.Copy)
        nc.scalar.activation(onescB[:, 1:2], ones16f[:, 0:1], AF.Copy,
                             scale=1.0 / C)
        onesrf = cp.tile([1, 128], F32)
        nc.vector.memset(onesrf[0:1, :], 1.0)
        ones_row = cp.tile([1, 128], BF16)
        nc.scalar.activation(ones_row[0:1, :], onesrf[0:1, :], AF.Copy)

        # Two LayerNorm stats (LN1 of block t+1, LN2 of block t-1) that land
        # in the same pipeline iteration share one PSUM tile and one set of
        # tail ops on [2, 256] — halves the ACT/DVE small-op overhead.
        stats_ctx = {"tile": None, "pend": []}

        def _ln_pre(src, st, key, tag, expect_pair=False):
            """Pre-reduce src and matmul [sum | sumsq] into a shared row."""
            if stats_ctx["tile"] is None:
                ps_ln = pp.tile([128, 512], F32, tag="ps")
                stats_ctx["tile"] = ps_ln
            row = len(stats_ctx["pend"])
            sq = lnp.tile([128, KC, TOK], BF16, tag=f"sq{tag}")
            nc.vector.tensor_mul(sq[:], src[:], src[:])
            t4 = lnp.tile([128, 2, 2, TOK], BF16, tag=f"t4{tag}")
            nc.vector.tensor_add(t4[:, 0], src[:, 0:2, :], src[:, 2:4, :])
            nc.vector.tensor_add(t4[:, 1], sq[:, 0:2, :], sq[:, 2:4, :])
            red = lnp.tile([128, 2, TOK], BF16, tag=f"red{tag}")
            nc.vector.tensor_add(red[:], t4[:, :, 0, :], t4[:, :, 1, :])
            nc.tensor.matmul(stats_ctx["tile"][0:2, :],
                             onescA[:] if row == 0 else onescB[:],
                             red[:], start=(row == 0),
                             stop=(row == 1 or not expect_pair),
                             skip_group_check=True)
            stats_ctx["pend"].append((st, key))

        def _ln_flush():
            """Run the stats tail on all pending rows, stash rstd rows."""
            n = len(stats_ctx["pend"])
            if n == 0:
                return
            ps_ln = stats_ctx["tile"]
            m2 = lnp.tile([2, 256], F32, tag="m2")
            nc.scalar.activation(m2[0:n, :], ps_ln[0:n, 0:256], AF.Square)
            var = lnp.tile([2, 256], F32, tag="var")
            nc.vector.scalar_tensor_tensor(var[0:n, :], ps_ln[0:n, 256:512],
                                           EPS, m2[0:n, :],
                                           op0=ALU.add, op1=ALU.subtract)
            lnv = lnp.tile([2, 256], F32, tag="lnv")
            nc.scalar.activation(lnv[0:n, :], var[0:n, :], AF.Ln)
            rstd = lnp.tile([2, 256], BF16, tag="rstd")
            # rstd = exp(-0.5 * ln(var)) — keeps ACT on the exp/ln table.
            nc.scalar.activation(rstd[0:n, :], lnv[0:n, :], AF.Exp,
                                 scale=-0.5)
            for row, (st, key) in enumerate(stats_ctx["pend"]):
                st[key] = (rstd, row)
            stats_ctx["tile"] = None
            stats_ctx["pend"] = []

        def _ln_bcast(rstd_row, tag):
            """Broadcast an rstd row across 128 partitions (stride-0 DMA)."""
            rstd, row = rstd_row
            rb = sbe.tile([128, 256], BF16, tag=f"rb{tag}")
            nc.scalar.dma_start(rb[:], _bcast_part(rstd[row:row + 1, :], 128))
            return rb

        def _apply(src, rb, dst_tag):
            """xhat = src * rstd (weights are host-centered, no mean term)."""
            dst = sb.tile([128, KC, TOK], BF16, tag=dst_tag)
            nc.vector.tensor_mul(dst[:], src[:], _rep(rb[:, :], KC))
            return dst

        def pf_load(t):
            st = {"b": t}
            x_sb = sbe.tile([128, KC, TOK], BF16, tag="x_sb")
            nc.sync.dma_start(x_sb[:],
                              xs_d[t % NBLK].rearrange("a p b -> p a b"))
            st["x"] = x_sb
            return st

        def s0a_stats(st, expect_pair=False):
            _ln_pre(st["x"], st, "rstd1", "1", expect_pair=expect_pair)

        def s0b_bcast(st):
            st["rb1"] = _ln_bcast(st["rstd1"], "1")

        def s1_apply(st):
            st["xh"] = _apply(st["x"], st["rb1"], "xhat1")

        def s1_qkv(st):
            xh = st.pop("xh")
            qT = sb.tile([128, KC, TOK], BF16, tag="qT")
            kT = sb.tile([128, KC, TOK], BF16, tag="kT")
            for dst, w_s, bcol0, nz, dve in ((qT, wq_s, 0, bq_nz, True),
                                             (kT, wk_s, 4, bk_nz, False)):
                for pair in range(2):
                    ps_q = pp.tile([128, 512], F32, tag="ps")
                    for half in range(2):
                        mf = pair * 2 + half
                        o = ps_q[:, half * 256:(half + 1) * 256]
                        for kc in range(KC):
                            nc.tensor.matmul(
                                o, w_s[:, kc, mf * 128:(mf + 1) * 128],
                                xh[:, kc, :],
                                start=(kc == 0), stop=(kc == KC - 1))
                    if nz:
                        for half in range(2):
                            mf = pair * 2 + half
                            nc.scalar.activation(
                                dst[:, mf, :],
                                ps_q[:, half * 256:(half + 1) * 256],
                                AF.Identity, bias=br_s[:, bcol0 + mf:bcol0 + mf + 1])
                    elif dve:
                        nc.vector.tensor_copy(
                            dst[:, pair * 2:(pair + 1) * 2, :],
                            ps_q[:].rearrange("p (a b) -> p a b", a=2))
                    else:
                        nc.scalar.activation(
                            dst[:, pair * 2:(pair + 1) * 2, :],
                            ps_q[:].rearrange("p (a b) -> p a b", a=2), AF.Copy)
            v65 = sb.tile([128, 2, NH, 65], BF16, tag="v65")
            nc.scalar.activation(
                v65[:, :, :, 64:65],
                ones16b[:].rearrange("p (a h b) -> p a h b", a=2, h=NH), AF.Copy)
            for tcx in range(2):
                ps_v = pp.tile([128, 512], F32, tag="ps")
                for kc in range(KC):
                    nc.tensor.matmul(
                        ps_v[:], xh[:, kc, tcx * 128:(tcx + 1) * 128],
                        wv_s[:, kc, :],
                        start=(kc == 0), stop=(kc == KC - 1 and not bv_nz))
                if bv_nz:
                    nc.tensor.matmul(ps_v[:], ones_row[:], bvr_s[0:1, :],
                                     start=False, stop=True)
                if tcx == 0:
                    nc.vector.tensor_copy(
                        v65[:, tcx, :, 0:64],
                        ps_v[:].rearrange("p (h e) -> p h e", h=NH))
                else:
                    nc.scalar.activation(
                        v65[:, tcx, :, 0:64],
                        ps_v[:].rearrange("p (h e) -> p h e", h=NH), AF.Copy)
            st["qT"], st["kT"], st["v65"] = qT, kT, v65

        def _head(st, hh):
            """Scores (+ low-rank bias) -> exp. Returns e_t for the AV step."""
            qT, kT = st["qT"], st["kT"]
            mf, po = hh // 2, (hh % 2) * 64
            ps_s = pp.tile([128, 512], F32, tag="ps")
            for kt in range(2):
                nc.tensor.matmul(
                    ps_s[:, kt * 256:(kt + 1) * 256],
                    kT[po:po + 64, mf, kt * 128:(kt + 1) * 128],
                    qT[po:po + 64, mf, :], start=True, stop=False)
                nc.tensor.matmul(
                    ps_s[:, kt * 256:(kt + 1) * 256],
                    vt_s[:, hh, kt * 128:(kt + 1) * 128],
                    ut_s[:, hh, :], start=False, stop=True)
            e_t = lnp.tile([128, 2, TOK], BF16, tag="E")
            nc.scalar.activation(e_t[:],
                                 ps_s[:].rearrange("p (a b) -> p a b", a=2),
                                 AF.Exp)
            return e_t

        def _head_av(st, hh, e_t):
            v65 = st["v65"]
            if hh % 4 == 0:
                avg = psa.tile([65, 4, TOK], F32, tag="av")
                st[f"av{hh // 4}"] = avg
            avg = st[f"av{hh // 4}"]
            for kt in range(2):
                nc.tensor.matmul(avg[:, hh % 4, :], v65[:, kt, hh, :],
                                 e_t[:, kt, :],
                                 start=(kt == 0), stop=(kt == 1))
            if hh % 4 == 3:
                # Half-group denominator chain: Ln reads the ones-column row
                # straight from PSUM (the Ln IS the drain), a tiny DMA
                # spreads it to 4 partitions, one Exp gives 1/d. The first
                # half completes while heads 4-7 are still on the PE.
                g = hh // 4
                d8l = lnp.tile([1, 4, TOK], F32, tag="d8l")
                nc.scalar.activation(d8l[0:1, :, :],
                                     avg[64:65, :, :],
                                     AF.Ln)
                d8f = sbe.tile([4, TOK], F32, tag=f"d8f{g}")
                nc.scalar.dma_start(d8f[:], d8l[0:1, :, :])
                d8r = sbe.tile([4, TOK], BF16, tag=f"d8r{g}")
                nc.scalar.activation(d8r[:], d8f[:], AF.Exp, scale=-1.0)
                st[f"d8r{g}"] = d8r

        def _norm_group(st, g):
            """Broadcast group g's 1/denom via the sel matmul and normalize
            its 4 heads into aT chunks 2g..2g+1 (two [64, 2, TOK] DVE ops).
            Group 0 runs inside s2b, overlapped with heads 5-7 on the PE."""
            avg, rb8, aT = st[f"av{g}"], st["rb8"], st["aT"]
            d8r = st[f"d8r{g}"]
            ps_rb = pp.tile([128, 512], F32, tag="ps")
            for half in range(2):
                nc.tensor.matmul(ps_rb[:, half * 256:(half + 1) * 256],
                                 sel[0:4, half * 128:(half + 1) * 128],
                                 d8r[:, :], start=True, stop=True)
            nc.vector.tensor_copy(
                rb8[:, g * 2:(g + 1) * 2, :],
                ps_rb[:].rearrange("p (a b) -> p a b", a=2))
            c0, c1 = 2 * g, 2 * g + 2
            nc.vector.tensor_mul(aT[0:64, c0:c1, :],
                                 avg[0:64, 0:4:2, :],
                                 rb8[0:64, c0:c1, :])
            nc.vector.tensor_mul(aT[64:128, c0:c1, :],
                                 avg[0:64, 1:4:2, :],
                                 rb8[64:128, c0:c1, :])

        def s2a_attn(st):
            """Heads 0-3, software-pipelined by one head."""
            rb8 = sbe.tile([128, KC, TOK], BF16, tag="rb8")
            aT = sb.tile([128, KC, TOK], BF16, tag="aT")
            st["rb8"], st["aT"] = rb8, aT
            e_prev = _head(st, 0)
            for hh in range(1, 4):
                e_t = _head(st, hh)
                _head_av(st, hh - 1, e_prev)
                e_prev = e_t
            st["e_prev"] = e_prev

        def s2b_attn(st):
            e_prev = st.pop("e_prev")
            for hh in range(4, 8):
                e_t = _head(st, hh)
                _head_av(st, hh - 1, e_prev)
                e_prev = e_t
                if hh == 7:
                    # Group 0's normalize: its reciprocal chain finished
                    # while heads 5-6 were on the PE.
                    _norm_group(st, 0)
            _head_av(st, 7, e_prev)

        def s3a_norm_proj(st):
            _norm_group(st, 1)
            aT = st["aT"]
            o_sb = sbe.tile([128, KC, TOK], BF16, tag="o_sb")
            for pair in range(2):
                ps_o = pp.tile([128, 512], F32, tag="ps")
                for half in range(2):
                    mc = pair * 2 + half
                    o = ps_o[:, half * 256:(half + 1) * 256]
                    for fc in range(KC):
                        nc.tensor.matmul(
                            o, wp_s[:, fc, mc * 128:(mc + 1) * 128],
                            aT[:, fc, :],
                            start=(fc == 0), stop=(fc == KC - 1))
                nc.vector.tensor_add(
                    o_sb[:, pair * 2:(pair + 1) * 2, :],
                    ps_o[:].rearrange("p (a b) -> p a b", a=2),
                    st["x"][:, pair * 2:(pair + 1) * 2, :])
            st["o"] = o_sb

        def s3b1_stats(st):
            _ln_pre(st["o"], st, "rstd2", "2")

        def s3b2_bcast(st):
            st["rb2"] = _ln_bcast(st["rstd2"], "2")

        def s4a_apply(st):
            st["yh"] = _apply(st["o"], st["rb2"], "xhat2")

        def s4a_ffn1(st):
            yh = st.pop("yh")
            h1 = sb.tile([128, KC, TOK], BF16, tag="h1")
            for pair in range(2):
                ps_h = pp.tile([128, 512], F32, tag="ps")
                for half in range(2):
                    mf = pair * 2 + half
                    o = ps_h[:, half * 256:(half + 1) * 256]
                    for kc in range(KC):
                        nc.tensor.matmul(
                            o, w1_s[:, kc, mf * 128:(mf + 1) * 128],
                            yh[:, kc, :],
                            start=(kc == 0), stop=(kc == KC - 1))
                if b1_nz:
                    for half in range(2):
                        mf = pair * 2 + half
                        nc.scalar.activation(
                            h1[:, mf, :], ps_h[:, half * 256:(half + 1) * 256],
                            AF.Relu, bias=br_s[:, 8 + mf:8 + mf + 1])
                else:
                    nc.scalar.activation(
                        h1[:, pair * 2:(pair + 1) * 2, :],
                        ps_h[:].rearrange("p (a b) -> p a b", a=2), AF.Relu)
            st["h1"] = h1

        def s4b_ffn2(st):
            o_sb, h1 = st["o"], st["h1"]
            out_sb = sb.tile([128, KC, TOK], F32, tag="out_sb")
            for pair in range(2):
                ps_y = pp.tile([128, 512], F32, tag="ps")
                for half in range(2):
                    mc = pair * 2 + half
                    o = ps_y[:, half * 256:(half + 1) * 256]
                    for fc in range(KC):
                        nc.tensor.matmul(
                            o, w2_s[:, fc, mc * 128:(mc + 1) * 128],
                            h1[:, fc, :],
                            start=(fc == 0), stop=(fc == KC - 1))
                if b2_nz:
                    for half in range(2):
                        mc = pair * 2 + half
                        nc.vector.scalar_tensor_tensor(
                            out_sb[:, mc, :],
                            ps_y[:, half * 256:(half + 1) * 256],
                            br_s[:, 12 + mc:12 + mc + 1],
                            o_sb[:, mc, :], op0=ALU.add, op1=ALU.add)
                else:
                    nc.vector.tensor_add(
                        out_sb[:, pair * 2:(pair + 1) * 2, :],
                        ps_y[:].rearrange("p (a b) -> p a b", a=2),
                        o_sb[:, pair * 2:(pair + 1) * 2, :])
            nc.gpsimd.dma_start(out_d[st["b"] % NBLK].rearrange("a p b -> p a b"),
                                out_sb[:])

        # Software pipeline across blocks. PE order per iteration:
        # scores/AV(t-1) -> QKV(t) -> ffn1(t-2) -> selMM+proj(t-1) ->
        # ffn2(t-2): the t-1 softmax-denominator chain (ACT/DMA/DVE) hides
        # under ~6us of QKV+ffn1 matmuls before proj needs aT.
        blocks = {}
        NT = NBLK * repeat
        blocks[0] = pf_load(0)
        s0a_stats(blocks[0])
        _ln_flush()
        s0b_bcast(blocks[0])
        for t in range(NT):
            if t - 1 >= 0:
                s2a_attn(blocks[t - 1])
            s1_apply(blocks[t])
            if t - 2 >= 0:
                s4a_apply(blocks[t - 2])
            if t - 1 >= 0:
                s2b_attn(blocks[t - 1])
            s1_qkv(blocks[t])
            if t - 2 >= 0:
                s4a_ffn1(blocks[t - 2])
            if t - 1 >= 0:
                s3a_norm_proj(blocks[t - 1])
            if t + 1 < NT:
                blocks[t + 1] = pf_load(t + 1)
                s0a_stats(blocks[t + 1], expect_pair=(t - 1 >= 0))
            if t - 1 >= 0:
                s3b1_stats(blocks[t - 1])
            _ln_flush()
            if t + 1 < NT:
                s0b_bcast(blocks[t + 1])
            if t - 2 >= 0:
                s4b_ffn2(blocks.pop(t - 2))
            if t - 1 >= 0:
                s3b2_bcast(blocks[t - 1])
        tl = NT - 1
        s2a_attn(blocks[tl])
        s4a_apply(blocks[tl - 1])
        s2b_attn(blocks[tl])
        s4a_ffn1(blocks[tl - 1])
        s3a_norm_proj(blocks[tl])
        s3b1_stats(blocks[tl])
        _ln_flush()
        s4b_ffn2(blocks.pop(tl - 1))
        s3b2_bcast(blocks[tl])
        s4a_apply(blocks[tl])
        s4a_ffn1(blocks[tl])
        s4b_ffn2(blocks.pop(tl))

    _legalize_waits(nc)
    return nc


_CACHE = {}


def _get_nc(flags, repeat=1):
    key = (flags, repeat)
    if key not in _CACHE:
        _CACHE[key] = build_kernel(*flags, repeat=repeat)
    return _CACHE[key]


def _axial_bias_np(dt_bank, dh_bank, dw_bank):
    ar = np.arange(THW)
    tt = ar // (BH * BW)
    hh = (ar // BW) % BH
    ww = ar % BW
    it = tt[:, None] - tt[None, :] + (BT - 1)
    ih = hh[:, None] - hh[None, :] + (BH - 1)
    iw = ww[:, None] - ww[None, :] + (BW - 1)
    return dt_bank[:, it] + dh_bank[:, ih] + dw_bank[:, iw]  # (NH, 256, 256)


def prepare(x, dt_bank, dh_bank, dw_bank, ln1_g, ln1_b, w_q, w_k, w_v,
            w_proj, ln2_g, ln2_b, w1, b1, w2, b2):
    """Host-side prep: returns (flags, in_maps)."""
    f = np.float32
    x = np.asarray(x, f)

    # block split: (B,C,T,H,W) -> (NB, C, THW), channels-major
    xb = x.reshape(B, C, ST, BT, SH, BH, SW, BW)
    xb = xb.transpose(0, 2, 4, 6, 1, 3, 5, 7).reshape(NB, C, THW)
    xb = np.ascontiguousarray(xb).reshape(NB, KC, 128, TOK).astype(BF)

    scale = 1.0 / np.sqrt(DA)
    wqf = np.asarray(w_q, f).transpose(1, 0, 2).reshape(C, NH * DA)
    wkf = np.asarray(w_k, f).transpose(1, 0, 2).reshape(C, NH * DA)
    wvf = np.asarray(w_v, f).transpose(1, 0, 2).reshape(C, NH * DA)
    g1 = np.asarray(ln1_g, f)[:, None]
    b1v = np.asarray(ln1_b, f)
    # Center columns (over the contract dim): W~^T x == W^T (x - mean(x)).
    wq_c = (g1 * wqf) * scale
    wk_c = g1 * wkf
    wv_c = g1 * wvf
    wq_c = wq_c - wq_c.mean(axis=0, keepdims=True)
    wk_c = wk_c - wk_c.mean(axis=0, keepdims=True)
    wv_c = wv_c - wv_c.mean(axis=0, keepdims=True)
    wq_e = np.ascontiguousarray(wq_c).reshape(KC, 128, 512)
    wk_e = np.ascontiguousarray(wk_c).reshape(KC, 128, 512)
    wv_e = np.ascontiguousarray(wv_c).reshape(KC, 128, 512)
    bq = (b1v @ wqf) * scale
    bk = b1v @ wkf
    bv = b1v @ wvf
    wp_e = np.ascontiguousarray(np.asarray(w_proj, f).T).reshape(KC, 128, 512)
    g2 = np.asarray(ln2_g, f)[:, None]
    b2v = np.asarray(ln2_b, f)
    w1t = np.asarray(w1, f).T
    w1_c = g2 * w1t
    w1_c = w1_c - w1_c.mean(axis=0, keepdims=True)
    w1_e = np.ascontiguousarray(w1_c).reshape(KC, 128, 512)
    b1p = b2v @ w1t + np.asarray(b1, f)
    w2_e = np.ascontiguousarray(np.asarray(w2, f).T).reshape(KC, 128, 512)
    b2p = np.asarray(b2, f)

    brows = np.zeros((128, 16), f)
    brows[:, 0:4] = bq.reshape(KC, 128).T
    brows[:, 4:8] = bk.reshape(KC, 128).T
    brows[:, 8:12] = b1p.reshape(KC, 128).T
    brows[:, 12:16] = b2p.reshape(KC, 128).T
    bvrow = np.ascontiguousarray(bv.reshape(1, 512)).astype(BF)

    # Low-rank factorization of the (transposed) axial bias: B^T = V @ U^T,
    # exact rank <= 4+8+8 = 20, accumulated into the scores matmul on PE.
    bias = _axial_bias_np(np.asarray(dt_bank, f), np.asarray(dh_bank, f),
                          np.asarray(dw_bank, f))
    R = 20
    ut = np.zeros((NH, R, THW), np.float64)
    vt = np.zeros((NH, R, THW), np.float64)
    for h in range(NH):
        U, S, Vh = np.linalg.svd(bias[h].T.astype(np.float64))
        r = min(R, int((S > S[0] * 1e-7).sum()) if S[0] > 0 else 1)
        vt[h, :r] = (U[:, :r] * np.sqrt(S[:r])).T
        ut[h, :r] = Vh[:r] * np.sqrt(S[:r])[:, None]
    ut = np.ascontiguousarray(ut).astype(BF)
    vt = np.ascontiguousarray(vt).astype(BF)

    flags = (bool(bq.any()), bool(bk.any()), bool(bv.any()),
             bool(b1p.any()), bool(b2p.any()))

    # sel[j, half*128 + p] = 1 iff 2*half + p//64 == j (j < 4): broadcasts a
    # 4-head group's 1/denom rows to all 128 partitions of a channel chunk.
    selm = np.zeros((NH, KC * 128), f)
    for half in range(2):
        for p in range(128):
            selm[2 * half + p // 64, half * 128 + p] = 1.0

    shared = {"wq": wq_e.astype(BF), "wk": wk_e.astype(BF),
              "wv": wv_e.astype(BF), "wp": wp_e.astype(BF),
              "w1": w1_e.astype(BF), "w2": w2_e.astype(BF),
              "ut": ut, "vt": vt, "brows": brows, "bvrow": bvrow,
              "sel": selm.astype(BF)}
    in_maps = []
    for i in range(NCORES):
        m = dict(shared)
        m["xs"] = np.ascontiguousarray(xb[i * NBLK:(i + 1) * NBLK])
        in_maps.append(m)
    return flags, in_maps


def gather(results):
    outs = np.concatenate([results[i]["out"][None] for i in range(NCORES)])
    # (NCORES, NBLK, KC, 128, TOK) -> (NB, C, THW) -> (B, C, T, H, W)
    ob = outs.reshape(NB, C, THW)
    ob = ob.reshape(B, ST, SH, SW, C, BT, BH, BW)
    ob = ob.transpose(0, 4, 1, 5, 2, 6, 3, 7).reshape(B, C, T, H, W)
    return np.ascontiguousarray(ob)


def kernel(**inputs):
    from concourse.bass_utils import run_bass_kernel_spmd

    flags, in_maps = prepare(**inputs)
    nc = _get_nc(flags)
    res = run_bass_kernel_spmd(nc, in_maps, list(range(NCORES)))
    return gather(res.results)
